# revision 1
# baseline (speedup 1.0000x reference)
"""CRF Viterbi decode kernel for Trainium2 (Bass), data-parallel over batch.

Problem shapes (hardcoded): X [32,128,10000] f32 one-hot, t_feats [48,48],
e_feats [48,10000].  Output Y_hat [32,128,48] f32 one-hot.

Sharding: batch 32 -> 8 cores x 4.  t_feats / e_feats replicated.

Per-core pipeline (4 batch elems as 2 chains of 2 elems on 96 partitions):
  1. emissions em[(b,l), t] = e_feats @ X_b^T per 128-wide V chunk: DMA X
     naturally [t,(b,v)], PE-transpose (f32r) to [v,t], matmul accumulate in
     PSUM with an exact f32r hi+lo split of e_feats.  The lhsT free dim is
     broadcast to 96 so both batch elems of a chain land on partition blocks
     0:48 / 48:96 directly.
  2. forward Viterbi scan over t: scores psc[(b,j),(b,i)] kept resident in
     PSUM and updated each step with ONE broadcast-transpose matmul of
     delta_t - delta_{t-1} (computed by a fused scalar_tensor_tensor on DVE);
     DVE max8 -> m8 slab.  Periodic full refresh bounds fp drift.  No
     backpointers / max_index needed.
  3. backward "end-anytime" scan (runs concurrently): psb = t2bd + bcast(em)
     + bcast(beta_{n+1}) rebuilt per step in alternating PSUM banks (the two
     static matmuls pre-execute); Pool (gpsimd) running-max scan with
     initial=0 implements beta_n = max(0, max_k ...) and writes a sliding
     window whose last column lands in a compact beta slab.
  4. combine: labels y_n = onehot(argmax_j delta_n(j) + beta_n(j)) * [n <=
     end_n] -- all bulk tensor ops + small transposes; no serial backtrace.
"""

import os
import sys

import numpy as np

for _p in ("/opt/trn_rl_repo",):
    if _p not in sys.path and os.path.isdir(_p):
        sys.path.insert(0, _p)

import concourse.bass as bass
import concourse.tile as tile
from concourse import mybir
from concourse.bass_utils import run_bass_kernel_spmd

F32 = mybir.dt.float32
F32R = mybir.dt.float32r
U32 = mybir.dt.uint32
AL = mybir.AluOpType

B, T, V, L = 32, 128, 10000, 48
NCORES = 8
BLOC = B // NCORES          # 4 batch elems per core
NCH = 2                     # chains per core (2 batch elems each)
P2 = 2 * L                  # 96 partitions per chain
NEG = -1.0e30               # block-diagonal mask value
NV0 = 78 * 128              # bulk V rows (chunks 0..77)
VCH = [128] * 78 + [16]     # V chunking (sum = 10000)
REFRESH = 16                # fwd scan full-refresh period (fp drift bound)

# packed constants layout: [128, CW] f32
# cols 0:128    ident (128 partitions)
# cols 128:224  blockdiag(t_feats) fwd (96 partitions)
# cols 224:320  blockdiag(t_feats^T) bwd (96 partitions)
# col  320      d0 column (96 partitions)
# cols 321:450  iota129 (4 partitions)
# cols 450:498  iota48 (128 partitions)
CW = 498


def build_nc():
    nc = bass.Bass()

    x = nc.declare_dram_parameter("x", [BLOC, T, V], F32R, isOutput=False)
    eT = nc.declare_dram_parameter("eT", [128, 78 * L + L], F32, isOutput=False)
    consts = nc.declare_dram_parameter("consts", [128, CW], F32, isOutput=False)
    y = nc.declare_dram_parameter("y", [BLOC, T, L], F32, isOutput=True)

    with tile.TileContext(nc) as tc:
        from contextlib import ExitStack

        with ExitStack() as ctx:
            cons = ctx.enter_context(tc.tile_pool(name="cons", bufs=1))
            pers = ctx.enter_context(tc.tile_pool(name="pers", bufs=1))
            xpool = ctx.enter_context(tc.tile_pool(name="xpool", bufs=3))
            xtpool = ctx.enter_context(tc.tile_pool(name="xtpool", bufs=3))
            srpool = ctx.enter_context(tc.tile_pool(name="srpool", bufs=2))
            ppxt = ctx.enter_context(tc.tile_pool(name="ppxt", bufs=2, space="PSUM"))
            ppem = ctx.enter_context(tc.tile_pool(name="ppem", bufs=1, space="PSUM"))
            ppsc = ctx.enter_context(tc.tile_pool(name="ppsc", bufs=1, space="PSUM"))
            ppsb = ctx.enter_context(tc.tile_pool(name="ppsb", bufs=3, space="PSUM"))

            # ---- constants: ONE DMA ----
            cons_sb = cons.tile([128, CW], F32)
            nc.sync.dma_start(out=cons_sb, in_=consts[:, :])
            id_sb = cons_sb[:, 0:128]
            id96 = cons_sb[0:P2, 0:P2]
            tbdf_sb = cons_sb[0:P2, 128:224]
            tbdb_sb = cons_sb[0 : P2 + 1, 224:320]
            id97 = cons_sb[0 : P2 + 1, 0 : P2 + 1]
            d0_sb = cons_sb[0:P2, 320:321]
            io129_sb = cons_sb[0:BLOC, 321:450]
            io48_sb = cons_sb[:, 450:498]
            # f32r-rounded identity (values 0/1: rounding is exact); the BIR
            # verifier requires f32r matmult inputs to come from a rounding op
            idr_t = cons.tile([128, 128], F32)
            nc.scalar.copy(out=idr_t.bitcast(F32R), in_=id_sb)
            idr = idr_t.bitcast(F32R)

            # ---- e_feats^T staged, split into exact f32r hi+lo terms.
            # The host pre-permutes eT into the SBUF layout ([v%128, k, l] with
            # the 16-row tail in the last L cols) so ONE contiguous DMA with
            # 15KB elements replaces the 2x-penalized 192B-element rearrange.
            efr = cons.tile([128, 78 * L + L], F32)
            nc.sync.dma_start(out=efr, in_=eT[:, :])
            view = efr[:, : 78 * L].rearrange("v (k l) -> v k l", l=L)
            ef_hi = cons.tile([128, 78, L], F32)
            ef_lo = cons.tile([128, 78, L], F32)
            for h in range(2):
                hs = slice(h * 39, (h + 1) * 39)
                nc.scalar.copy(
                    out=ef_hi[:, hs, :].bitcast(F32R), in_=view[:, hs, :]
                )
                nc.vector.tensor_sub(
                    ef_lo[:, hs, :].bitcast(F32R),
                    view[:, hs, :],
                    ef_hi[:, hs, :],
                )
            ef_tail_raw = efr[0:16, 78 * L : 78 * L + L]
            ef_hi_t = cons.tile([16, L], F32)
            nc.scalar.copy(out=ef_hi_t.bitcast(F32R), in_=ef_tail_raw)
            ef_lo_t = cons.tile([16, L], F32)
            nc.vector.tensor_sub(ef_lo_t.bitcast(F32R), ef_tail_raw, ef_hi_t)

            # ---- persistent state ----
            m8slab = [pers.tile([P2, 8 * T], F32, name=f"m8_{c}") for c in range(NCH)]
            em_sb = [
                pers.tile([P2 + 1, T], F32, name=f"em_{c}") for c in range(NCH)
            ]
            demf = [pers.tile([P2, T - 2], F32, name=f"demf_{c}") for c in range(NCH)]
            # compact beta slab: rows 0:96 = beta, row 96 = static 0 (the
            # broadcast matmuls read [97,1] columns incl. the zero row)
            bslab = [
                pers.tile([P2 + 1, T + 1], F32, name=f"bs_{c}") for c in range(NCH)
            ]
            dcols = [pers.tile([P2, T + 1], F32, name=f"dc_{c}") for c in range(NCH)]
            fcols = [pers.tile([P2, T + 1], F32, name=f"fc_{c}") for c in range(NCH)]

            # fwd score tiles: one full PSUM bank each (never closed)
            psc = [ppsc.tile([P2, 512], F32, name=f"psc_{c}") for c in range(NCH)]
            for c in range(NCH):
                nc.vector.memset(bslab[c][:, T : T + 1], 0.0)      # beta_T = 0
                nc.vector.memset(bslab[c][P2 : P2 + 1, :], 0.0)    # zero row
                nc.vector.memset(em_sb[c][P2 : P2 + 1, :], 0.0)    # zero row

            # ---- emissions: pass per chain ----
            nchunks = len(VCH)
            QBOUND = [0, 8, 16, 24, 32, 40, 48, 56, 64, 72, nchunks]

            def emit_pass(c):
                bs = [2 * c, 2 * c + 1]
                pem = ppem.tile([L, 512], F32, name="pem", tag="pem")
                pemv = pem[:, : 2 * T].rearrange("p (b t) -> p b t", b=2)
                vo = 0
                for q in range(len(QBOUND) - 1):
                    k0q, k1q = QBOUND[q], QBOUND[q + 1]
                    qo = k0q * 128
                    qw = sum(VCH[k0q:k1q])
                    xbs = []
                    for bb, b in enumerate(bs):
                        xb = xpool.tile([T, 2560], F32R, name="xb", tag=f"xb{bb}")
                        nc.sync.dma_start(out=xb[:, :qw], in_=x[b, :, qo : qo + qw])
                        xbs.append(xb)
                    klist = list(range(k0q, k1q))
                    for kk0 in range(0, len(klist), 2):
                        kpair = klist[kk0 : kk0 + 2]
                        pxt = ppxt.tile([128, 2, 2, T], F32, name="pxt")
                        pxtr = pxt.bitcast(F32R)
                        for kj, k in enumerate(kpair):
                            vs = VCH[k]
                            ko = vo - qo
                            for bb in range(2):
                                nc.tensor.transpose(
                                    pxtr[:vs, kj, bb, :],
                                    xbs[bb][:, ko : ko + vs],
                                    idr,
                                )
                            vo += vs
                        xt = xtpool.tile([128, 2, 2, T], F32, name="xt")
                        use_dve = False
                        if len(kpair) == 2 and min(VCH[k] for k in kpair) == 128:
                            if use_dve:
                                nc.vector.tensor_copy(
                                    xt.bitcast(F32R), pxt.bitcast(F32R)
                                )
                            else:
                                nc.scalar.copy(out=xt.bitcast(F32R), in_=pxt)
                        else:
                            for kj, k in enumerate(kpair):
                                nc.scalar.copy(
                                    out=xt[: VCH[k], kj, :, :].bitcast(F32R),
                                    in_=pxt[: VCH[k], kj, :, :],
                                )
                        for kj, k in enumerate(kpair):
                            vs = VCH[k]
                            ef_h = ef_hi[:, k, :] if k < 78 else ef_hi_t
                            ef_l = ef_lo[:, k, :] if k < 78 else ef_lo_t
                            rhs = (
                                xt[:vs, kj, :, :]
                                .rearrange("v b t -> v (b t)")
                                .bitcast(F32R)
                            )
                            nc.tensor.matmul(
                                pem[:, : 2 * T],
                                ef_h[:vs, :].bitcast(F32R),
                                rhs,
                                start=(k == 0),
                                stop=False,
                            )
                            nc.tensor.matmul(
                                pem[:, : 2 * T],
                                ef_l[:vs, :].bitcast(F32R),
                                rhs,
                                start=False,
                                stop=(k == nchunks - 1),
                            )
                        yield
                # stage emissions: PSUM -> SBUF copy, then partition-block
                # move via SBUF->SBUF DMA (engines cannot cross partitions)
                emt = srpool.tile([L, 2, T], F32, name="emt", tag="emt")
                nc.scalar.copy(out=emt, in_=pemv)
                for bb in range(2):
                    nc.sync.dma_start(
                        out=em_sb[c][bb * L : (bb + 1) * L, :],
                        in_=emt[:, bb, :],
                    )
                nc.vector.tensor_sub(
                    demf[c],
                    em_sb[c][0:P2, 1 : T - 1],
                    em_sb[c][0:P2, 0 : T - 2],
                )

            # ---- scan step emitters ----
            def fwd_step(c, p):
                """Emit fwd scan step p (producing delta_p's pre-em max m8[p-1])."""
                ps = psc[c][:, :P2]
                if p == 1:
                    nc.tensor.matmul(
                        ps, tbdf_sb, id96, start=True, stop=False,
                        is_transpose=True, skip_group_check=True,
                    )
                    nc.tensor.matmul(
                        ps, d0_sb.broadcast_to([P2, P2]), id96,
                        start=False, stop=False, is_transpose=True,
                        skip_group_check=True,
                    )
                elif p == 2 or (p - 2 - 8 * c) % REFRESH == 0:
                    # full refresh: psc = tbd + bcast(m8_{p-1} + em[p-2])
                    nc.tensor.matmul(
                        ps, tbdf_sb, id96, start=True, stop=False,
                        is_transpose=True, skip_group_check=True,
                    )
                    nc.tensor.matmul(
                        ps,
                        m8slab[c][:, 8 * (p - 2) : 8 * (p - 2) + 1]
                        .broadcast_to([P2, P2]),
                        id96, start=False, stop=False, is_transpose=True,
                        skip_group_check=True,
                    )
                    nc.tensor.matmul(
                        ps,
                        em_sb[c][0:P2, p - 2 : p - 1].broadcast_to([P2, P2]),
                        id96, start=False, stop=False, is_transpose=True,
                        skip_group_check=True,
                    )
                else:
                    # delta update: dd = (m8_{p-1} + (em[p-2]-em[p-3])) - m8_{p-2}
                    dd = srpool.tile([P2, 1], F32, name="dd", tag=f"dd{c}")
                    nc.vector.scalar_tensor_tensor(
                        out=dd,
                        in0=m8slab[c][:, 8 * (p - 2) : 8 * (p - 2) + 1],
                        scalar=demf[c][:, p - 3 : p - 2],
                        in1=m8slab[c][:, 8 * (p - 3) : 8 * (p - 3) + 1],
                        op0=AL.add,
                        op1=AL.subtract,
                    )
                    nc.tensor.matmul(
                        ps, dd.broadcast_to([P2, P2]), id96,
                        start=False, stop=False, is_transpose=True,
                        skip_group_check=True,
                    )
                nc.vector.max(m8slab[c][:, 8 * (p - 1) : 8 * (p - 1) + 8], ps)

            def bwd_static(c, pos):
                """Emit the dependency-free part of bwd step pos (pre-executes
                on PE while the rest of the pipeline catches up)."""
                pb = ppsb.tile([P2, 512], F32, name="pb", tag="pb")[:, : P2 + 1]
                nc.tensor.matmul(
                    pb, tbdb_sb, id97, start=True, stop=False, is_transpose=True,
                )
                last = pos == T - 1
                nc.tensor.matmul(
                    pb,
                    em_sb[c][:, pos : pos + 1].broadcast_to([P2 + 1, P2]),
                    id97, start=False, stop=last, is_transpose=True,
                )
                return pb

            def bwd_dyn_mm(c, pos, pb):
                if pos != T - 1:
                    nc.tensor.matmul(
                        pb,
                        bslab[c][:, pos + 1 : pos + 2].broadcast_to([P2 + 1, P2]),
                        id97, start=False, stop=True, is_transpose=True,
                    )

            def bwd_reduce(c, pos, pb):
                # beta_pos = max over 97 cols; col 96 == 0 supplies the floor
                nc.vector.tensor_reduce(
                    out=bslab[c][0:P2, pos : pos + 1],
                    in_=pb,
                    axis=mybir.AxisListType.X,
                    op=AL.max,
                )

            # ---- phase 1: emissions chain 0 ----
            for _ in emit_pass(0):
                pass

            # ---- phase 2: emissions chain 1 (full rate) interleaved with
            # scans chain 0; chain-1 scans join as soon as emissions land.
            # Program order = engine issue order, so chain-1 scan steps are
            # interleaved with chain-0's remaining steps (independent chains
            # pipeline across engines).
            for _ in emit_pass(1):
                pass

            # all scans post-emissions, both chains interleaved per step
            for t in range(T):
                pos = T - 1 - t
                p = t + 1
                for c in range(NCH):
                    pb = bwd_static(c, pos)
                    fwd_step(c, p)
                    bwd_dyn_mm(c, pos, pb)
                    bwd_reduce(c, pos, pb)

            # ---- phase 4: combine ----
            for c in range(NCH):
                nc.vector.tensor_copy(dcols[c][:, 0:1], d0_sb)
                nc.vector.tensor_add(
                    dcols[c][:, 1 : T + 1],
                    m8slab[c].rearrange("p (t e) -> p t e", e=8)[:, :, 0],
                    em_sb[c][0:P2, :],
                )
                nc.vector.tensor_add(fcols[c], dcols[c], bslab[c][0:P2, :])

            nm8 = pers.tile([T, 4 * 8], F32)
            fm8 = pers.tile([T, 4 * 8], F32)
            fi8 = pers.tile([T, 4 * 8], U32)
            fidx = pers.tile([T, 4], F32)
            for c in range(NCH):
                pdt = ppsb.tile([T, 512], F32, name="pdt", tag="pb")[:, :P2]
                nc.tensor.transpose(pdt, dcols[c][:, 1 : T + 1], id96)
                pft = ppem.tile([T, 512], F32, name="pft", tag="pem")[:, :P2]
                nc.tensor.transpose(pft, fcols[c][:, 1 : T + 1], id96)
                for bb in range(2):
                    k = 2 * c + bb
                    nc.vector.max(
                        nm8[:, 8 * k : 8 * k + 8], pdt[:, bb * L : (bb + 1) * L]
                    )
                    nc.vector.max(
                        fm8[:, 8 * k : 8 * k + 8], pft[:, bb * L : (bb + 1) * L]
                    )
                    nc.vector.max_index(
                        fi8[:, 8 * k : 8 * k + 8],
                        fm8[:, 8 * k : 8 * k + 8],
                        pft[:, bb * L : (bb + 1) * L],
                    )
            nc.vector.tensor_copy(
                fidx, fi8.rearrange("p (k e) -> p k e", e=8)[:, :, 0]
            )

            # n_maxs -> end_n (col 0 of delta_full maxes to 0 at label 0)
            pnm = ppxt.tile([BLOC, 2, T], F32, name="pnm", tag="pxt")
            nc.tensor.transpose(
                pnm[:, 0, :], nm8.rearrange("p (k e) -> p k e", e=8)[:, :, 0], id_sb
            )
            nmb = pers.tile([BLOC, T + 1], F32)
            nc.vector.memset(nmb[:, 0:1], 0.0)
            nc.scalar.copy(out=nmb[:, 1:], in_=pnm[:, 0, :])
            en8 = pers.tile([BLOC, 8], F32)
            nc.vector.max(en8, nmb)
            eni8 = pers.tile([BLOC, 8], U32)
            nc.vector.max_index(eni8, en8, nmb)
            endf = pers.tile([BLOC, 1], F32)
            nc.vector.tensor_copy(endf, eni8[:, 0:1])

            # active mask act[b, n] = (n <= end_n), transposed to [T, 4]
            act = pers.tile([BLOC, T + 1], F32)
            nc.vector.tensor_scalar(
                out=act, in0=io129_sb, scalar1=endf, scalar2=None, op0=AL.is_le
            )
            pact = ppxt.tile([T, BLOC], F32, name="pact", tag="pxt")
            nc.tensor.transpose(pact, act[:, 1:], id_sb[0:BLOC, 0:BLOC])
            actT = pers.tile([T, BLOC], F32)
            nc.scalar.copy(out=actT, in_=pact)

            # y one-hots
            ybig = pers.tile([T, BLOC, L], F32)
            for k in range(BLOC):
                nc.vector.tensor_scalar(
                    out=ybig[:, k, :],
                    in0=io48_sb[0:T, :],
                    scalar1=fidx[:, k : k + 1],
                    scalar2=actT[:, k : k + 1],
                    op0=AL.is_equal,
                    op1=AL.mult,
                )
            nc.sync.dma_start(
                out=y[:, :, :].rearrange("b t l -> t b l"), in_=ybig
            )

    nc.finalize()
    _legalize_sync_waits(nc)
    return nc


def _legalize_sync_waits(nc):
    """This container's walrus accepts at most ONE sync wait per instruction.

    Split excess waits onto Drain instructions inserted just before the
    offending instruction (same engine, so the waits still complete before it
    issues; an idle-pipe Drain costs ~12ns).  Applied to the serialized BIR
    only -- CoreSim consumes the in-memory module and is unaffected.
    """
    import json as _json

    m = _json.loads(nc.to_json_bytes())
    for f in m["functions"]:
        for blk in f["blocks"]:
            out = []
            for ins in blk["instructions"]:
                si = ins.get("sync_info") or {}
                w = si.get("on_wait") or []
                if len(w) > 1:
                    for j, wx in enumerate(w[:-1]):
                        out.append(
                            {
                                "debug": ins.get("debug", 0),
                                "engine": ins["engine"],
                                "ins": [],
                                "outs": [],
                                "name": f"{ins['name']}-w{j}",
                                "opcode": "Drain",
                                "sync_info": {"on_update": [], "on_wait": [wx]},
                            }
                        )
                    si["on_wait"] = [w[-1]]
                out.append(ins)
            blk["instructions"] = out
    blob = _json.dumps(m).encode()
    nc.to_json_bytes = lambda: blob


def make_consts():
    f32 = np.float32
    c = np.zeros((128, CW), f32)
    c[:128, 0:128] = np.eye(128, dtype=f32)
    c[0:P2, 128:320] = NEG
    c[0:P2, 320] = NEG
    c[0, 320] = 0.0
    c[L, 320] = 0.0
    c[0:BLOC, 321:450] = np.arange(T + 1, dtype=f32)[None, :]
    c[:, 450:498] = np.arange(L, dtype=f32)[None, :]
    return c


def make_in_maps(X, t_feats, e_feats):
    f32 = np.float32
    X = np.ascontiguousarray(X, dtype=f32)
    t_feats = np.asarray(t_feats, dtype=f32)
    e_feats = np.asarray(e_feats, dtype=f32)
    c = make_consts()
    # fwd scores matmul computes psc = tbdf.T @ id, so store blockdiag(t):
    # psc[(b,j),(b,i)] = tbdf[(b,i),(b,j)] = t[i,j]
    c[0:L, 128 : 128 + L] = t_feats
    c[L:P2, 128 + L : 224] = t_feats
    # bwd: psb[(b,j),(b,k)] = tbdb[(b,k),(b,j)] = t[j,k] -> store t^T blocks
    c[0:L, 224 : 224 + L] = t_feats.T
    c[L:P2, 224 + L : 320] = t_feats.T
    eTf = np.ascontiguousarray(e_feats.T)          # [V, L]
    blob = np.zeros((128, 78 * L + L), f32)
    bulk = eTf[: 78 * 128].reshape(78, 128, L)     # [k, p, L]
    blob[:, : 78 * L] = np.transpose(bulk, (1, 0, 2)).reshape(128, 78 * L)
    blob[0:16, 78 * L :] = eTf[78 * 128 :]
    eTm = blob
    in_maps = []
    for ci in range(NCORES):
        m = {
            "x": np.ascontiguousarray(X[ci * BLOC : (ci + 1) * BLOC]),
            "eT": eTm,
            "consts": c,
        }
        in_maps.append(m)
    return in_maps


_NC = None


def _get_nc():
    global _NC
    if _NC is None:
        _NC = build_nc()
    return _NC


def kernel(X, t_feats, e_feats):
    in_maps = make_in_maps(X, t_feats, e_feats)
    nc = _get_nc()
    res = run_bass_kernel_spmd(nc, in_maps, list(range(NCORES)))
    out = np.concatenate([res.results[c]["y"] for c in range(NCORES)], axis=0)
    return np.ascontiguousarray(out, dtype=np.float32)



# revision 18
# speedup vs baseline: 1.1137x; 1.1137x over previous
"""CRF Viterbi decode kernel for Trainium2 (Bass), data-parallel over batch.

Problem shapes (hardcoded): X [32,128,10000] f32 one-hot, t_feats [48,48],
e_feats [48,10000].  Output Y_hat [32,128,48] f32 one-hot.

Sharding: batch 32 -> 8 cores x 4.  t_feats / e_feats replicated.

Per-core pipeline (4 batch elems, labels on 48 partitions throughout):
  1. emissions em[l, (b,t)] = e_feats @ X^T: the host pre-transposes the
     one-hot X into an fp8 [v%128, v//128, (b t)] blob (0/1 are exact in
     fp8), so emissions are just 79 accumulating PE matmuls (f32r e-blob
     stationary x fp8 moving, 512-col outputs) into one PSUM bank -- no
     on-device transposes, no hi/lo split (f32r is exact f32 here), no
     staging copies beyond one PSUM->SBUF copy of the result.
  2. forward Viterbi: per batch elem a PSUM-resident score tile
     psc[j, i] = t[i,j] + delta[i], updated per step with two 48-col
     bf16-identity broadcast matmuls (~20ns each): on-path bcast(m_{p-1})
     and off-path bcast(demf[p-3] - m_{p-2}) (2-step slack).  DVE
     tensor_reduce (per pair of batch elems) produces m_p.  Periodic
     staggered full refreshes bound fp drift.
  3. backward "end-anytime" scan, same structure, with the max over
     next-labels done on the (otherwise idle) Pool engine via a
     mask-reset tensor_tensor_scan: state = max(mask*state, psb[k]);
     the mask zeroes at each batch elem's segment start, which also
     implements beta = max(0, .) for free.  The running-max row is
     written straight into the beta history slab (last col of each
     segment = the true beta).
  4. combine: y_n = onehot(argmax_j delta_n(j)+beta_n(j)) * [n <= end_n]
     via gpsimd partition_all_reduce + elementwise ops; no transposes.
     Output written label-major [L, B, T]; host unscrambles.
"""

import os
import sys

import numpy as np

for _p in ("/opt/trn_rl_repo",):
    if _p not in sys.path and os.path.isdir(_p):
        sys.path.insert(0, _p)

import concourse.bass as bass
import concourse.tile as tile
from concourse import mybir
from concourse.bass_utils import run_bass_kernel_spmd

F32 = mybir.dt.float32
F32R = mybir.dt.float32r
BF16 = mybir.dt.bfloat16
FP8 = mybir.dt.float8e4
AL = mybir.AluOpType
AX = mybir.AxisListType

B, T, V, L = 32, 128, 10000, 48
NCORES = 8
BLOC = B // NCORES          # 4 batch elems per core
NK = 79                     # V chunks of 128 (last one zero-padded)
BT = BLOC * T               # 512 moving columns, b-major
NEG = -1.0e30
P2 = 2 * L                  # 96

# consts layout [128, CW] f32:
#  cols 0:48     tfS   [96,48]  vstack(t, t)        (fwd refresh lhsT)
#  cols 48:96    tbr   [48,48]  t.T                 (bwd refresh lhsT)
#  col  96       d0stack [96,1]
#  col  97       d0col   [48,1]
#  cols 98:227   iota129 [48,129]
#  cols 227:275  iota48  [128,48]
CW = 275

# fwd refresh schedule: always at p==2 (clears the +-1e30 d0 arithmetic),
# then every 16 steps staggered per pair.  bwd staggered likewise on t.
RF = 16


def _fwd_refresh(p, pair):
    if p == 2:
        return True
    return p > 2 and (p - 2 - 8 * pair) % RF == 0


def _bwd_refresh(t, pair):
    if t == 0:
        return True
    return (t - 6 - 8 * pair) % RF == 0


def build_nc():
    nc = bass.Bass()

    x = nc.declare_dram_parameter("x", [128, NK, BT], BF16, isOutput=False)
    eT = nc.declare_dram_parameter("eT", [128, NK, 2, L], BF16, isOutput=False)
    consts = nc.declare_dram_parameter("consts", [128, CW], F32, isOutput=False)
    idb = nc.declare_dram_parameter("idb", [128, 128], F32, isOutput=False)
    y = nc.declare_dram_parameter("y", [T, BLOC, L], F32, isOutput=True)

    with tile.TileContext(nc) as tc:
        from contextlib import ExitStack

        with ExitStack() as ctx:
            cons = ctx.enter_context(tc.tile_pool(name="cons", bufs=1))
            pers = ctx.enter_context(tc.tile_pool(name="pers", bufs=1))
            ppem = ctx.enter_context(tc.tile_pool(name="ppem", bufs=1, space="PSUM"))
            ppfw = ctx.enter_context(tc.tile_pool(name="ppfw", bufs=1, space="PSUM"))
            ppbw = ctx.enter_context(tc.tile_pool(name="ppbw", bufs=1, space="PSUM"))
            ppcb = ctx.enter_context(tc.tile_pool(name="ppcb", bufs=1, space="PSUM"))

            # ---- constants ----
            cons_sb = cons.tile([128, CW], F32)
            nc.sync.dma_start(out=cons_sb, in_=consts[:, :])
            d0col = cons_sb[0:L, 97:98]
            io129 = cons_sb[0:L, 98:227]
            io48 = cons_sb[:, 227:275]

            tfr = cons_sb[0:P2, 0:L]
            tbr = cons_sb[0:L, L : 2 * L]
            d0r = cons_sb[0:P2, 96:97]

            idb_sb = cons.tile([128, 128], F32)
            nc.sync.dma_start(out=idb_sb, in_=idb[:, :])
            id96 = idb_sb[0:P2, 0:P2]
            id48 = idb_sb[0:L, 0:L]

            # ---- persistent state ----
            em48 = pers.tile([L, BLOC, T], F32, name="em48")
            demf = pers.tile([L, BLOC, T - 1], F32, name="demf")
            ndemf = pers.tile([L, BLOC, T - 1], F32, name="ndemf")
            mslab = pers.tile([L, BLOC, T + 1], F32, name="mslab")
            bslab = pers.tile([L, BLOC, T + 2], F32, name="bslab")
            ndm = pers.tile([L, 2, BLOC], F32, name="ndm")
            nbt = pers.tile([L, 2, BLOC], F32, name="nbt")
            dsl = pers.tile([L, BLOC, T + 1], F32, name="dsl")
            fsl = pers.tile([L, BLOC, T + 1], F32, name="fsl")
            fi8 = pers.tile([T, BLOC, 8], mybir.dt.uint32, name="fi8")
            fm8 = pers.tile([T, BLOC, 8], F32, name="fm8")
            fidx = pers.tile([T, BLOC], F32, name="fidx")
            nm = pers.tile([T, BLOC], F32, name="nm")
            nmb = pers.tile([BLOC, T + 1], F32, name="nmb")
            en8 = pers.tile([BLOC, 8], F32, name="en8")
            eni8 = pers.tile([BLOC, 8], mybir.dt.uint32, name="eni8")
            endf = pers.tile([BLOC, 1], F32, name="endf")
            act = pers.tile([BLOC, T + 1], F32, name="act")
            actT = pers.tile([T, BLOC], F32, name="actT")
            ybig = pers.tile([T, BLOC, L], F32, name="ybig")

            pscA = ppfw.tile([L, 2, 512], F32, name="pscA")
            psbA = ppbw.tile([L, 2, 512], F32, name="psbA")
            pem = ppem.tile([L, 512], F32, name="pem")
            # bwd uses 49-wide segments; cols 48/97 stay 0 forever (the
            # max-with-zero floor).  beta_T = 0 (bslab col T).
            nc.vector.memset(psbA[:, :, 48:105:56], 0.0)
            nc.vector.memset(bslab[:, :, T : T + 1], 0.0)

            def bmm(out, col, first=False, last=False, k96=False):
                """Broadcast col ([48,1] or [96,1]) along the free dim of out.

                lhsT/out are bitcast to f32r (same bits as f32 here) so the
                bf16 identity rhs keys the 1.0 cyc/row transpose path.
                """
                kk = P2 if k96 else L
                nc.tensor.matmul(
                    out,
                    col.broadcast_to([kk, L]),
                    id96 if k96 else id48,
                    start=first,
                    stop=last,
                    is_transpose=True,
                    skip_group_check=True,
                )

            def smm(out, lhsT, first=False):
                """Static 96-wide refresh matmul (t-term)."""
                nc.tensor.matmul(
                    out,
                    lhsT,
                    id96,
                    start=first,
                    stop=False,
                    is_transpose=True,
                    skip_group_check=True,
                )

            # ---- emissions: xt/ef live only in this phase; their pool is
            # closed afterwards so the beta slab can reuse the space ----
            epctx = tc.tile_pool(name="ep", bufs=1)
            ep = epctx.__enter__()
            ef = ep.tile([128, NK, 2, L], BF16)
            nc.sync.dma_start(out=ef, in_=eT[:, :, :, :])
            xt = ep.tile([128, NK, BT], BF16)
            KSL = [0, 10, 20, 30, 40, 50, 60, 70, NK]
            for s in range(len(KSL) - 1):
                k0, k1 = KSL[s], KSL[s + 1]
                nc.sync.dma_start(out=xt[:, k0:k1, :], in_=x[:, k0:k1, :])
                for k in range(k0, k1):
                    nc.tensor.matmul(
                        pem,
                        ef[:, k, 0, :],
                        xt[:, k, :],
                        start=(k == 0),
                        stop=False,
                    )
                    nc.tensor.matmul(
                        pem,
                        ef[:, k, 1, :],
                        xt[:, k, :],
                        start=False,
                        stop=(k == NK - 1),
                    )
            nc.scalar.copy(out=em48, in_=pem.rearrange("p (b t) -> p b t", b=BLOC))
            nc.vector.tensor_sub(demf, em48[:, :, 1:], em48[:, :, 0 : T - 1])
            nc.vector.tensor_sub(ndemf, em48[:, :, 0 : T - 1], em48[:, :, 1:])
            epctx.__exit__(None, None, None)  # xt/ef space no longer needed

            # ---- scans ----
            def fwd_step(t):
                p = t + 1
                # off-path bcast term (inputs >= 2 steps old), on Pool
                if p >= 3:
                    q = p % 2
                    nc.gpsimd.tensor_tensor(
                        ndm[:, q, :],
                        demf[:, :, p - 3],
                        mslab[:, :, p - 2],
                        op=AL.subtract,
                    )
                for pair in range(2):
                    ps = pscA[:, pair, :]
                    if p == 1:
                        smm(ps[:, 0:P2], tfr, first=True)
                        bmm(ps[:, 0:P2], d0r, k96=True)
                    elif _fwd_refresh(p, pair):
                        smm(ps[:, 0:P2], tfr, first=True)
                        for sub in range(2):
                            b = 2 * pair + sub
                            blk = ps[:, L * sub : L * sub + L]
                            bmm(blk, em48[:, b, p - 2 : p - 1])
                            bmm(blk, mslab[:, b, p - 1 : p])
                    else:
                        q = p % 2
                        for sub in range(2):
                            b = 2 * pair + sub
                            blk = ps[:, L * sub : L * sub + L]
                            bmm(blk, mslab[:, b, p - 1 : p])
                            bmm(blk, ndm[:, q, b : b + 1])
                nc.vector.tensor_reduce(
                    out=mslab[:, :, p : p + 1],
                    in_=pscA[:, :, 0:P2].rearrange("p a (s i) -> p a s i", s=2),
                    axis=AX.X,
                    op=AL.max,
                )

            def bwd_step(t):
                pos = T - 1 - t
                if pos <= T - 2:
                    # nb_pos = -demf[pos] - beta_{pos+2} (off-path, Pool)
                    q = t % 2
                    nc.gpsimd.tensor_tensor(
                        nbt[:, q, :],
                        ndemf[:, :, pos],
                        bslab[:, :, pos + 2],
                        op=AL.subtract,
                    )
                for pair in range(2):
                    pb = psbA[:, pair, :]
                    rf = _bwd_refresh(t, pair)
                    for sub in range(2):
                        b = 2 * pair + sub
                        blk = pb[:, 56 * sub : 56 * sub + L]
                        if rf:
                            nc.tensor.matmul(
                                blk,
                                tbr,
                                id48,
                                start=(sub == 0),
                                stop=False,
                                is_transpose=True,
                                skip_group_check=True,
                            )
                            bmm(blk, em48[:, b, pos : pos + 1])
                            if t > 0:
                                bmm(blk, bslab[:, b, pos + 1 : pos + 2])
                        else:
                            q = t % 2
                            bmm(blk, bslab[:, b, pos + 1 : pos + 2])
                            bmm(blk, nbt[:, q, b : b + 1])
                nc.vector.tensor_reduce(
                    out=bslab[:, :, pos : pos + 1],
                    in_=psbA[:, :, 0:112].rearrange("p a (s i) -> p a s i", i=56)[
                        :, :, :, 0:49
                    ],
                    axis=AX.X,
                    op=AL.max,
                )

            for t in range(T):
                fwd_step(t)
                bwd_step(t)

            # ---- combine ----
            id128f = idb_sb

            def ptrans(out_psum, in_sb):
                nc.tensor.matmul(
                    out_psum,
                    in_sb,
                    id128f[0 : in_sb.shape[0], 0 : in_sb.shape[0]],
                    start=True,
                    stop=True,
                    is_transpose=True,
                    skip_group_check=True,
                )

            # dsl: col0 = d0, cols 1: = m + em;  fsl = dsl + beta
            nc.vector.tensor_add(dsl[:, :, 1:], mslab[:, :, 1:], em48)
            nc.vector.tensor_copy(dsl[:, :, 0:1], d0col.broadcast_to([L, BLOC, 1]))
            nc.vector.tensor_add(fsl, dsl, bslab[:, :, 0 : T + 1])

            # per-b transposes to [T, 48] (fresh PSUM banks) + max/argmax
            pcb = ppcb.tile([128, 2, 512], F32, name="pcb")
            for b in range(BLOC):
                ptd = pcb[:, 0, b * L : (b + 1) * L][0:T, :]
                ptf = pcb[:, 1, b * L : (b + 1) * L][0:T, :]
                ptrans(ptd, dsl[:, b, 1:])
                ptrans(ptf, fsl[:, b, 1:])
                nc.vector.tensor_reduce(
                    out=nm[:, b : b + 1], in_=ptd, axis=AX.X, op=AL.max
                )
                nc.vector.max(fm8[:, b, :], ptf)
                nc.vector.max_index(fi8[:, b, :], fm8[:, b, :], ptf)
            nc.vector.tensor_copy(fidx, fi8[:, :, 0])

            # end_n per b from n_maxs (col 0 of delta_full maxes to 0)
            pnm = pcb[0:BLOC, 1, 192 : 192 + T]
            ptrans(pnm, nm)
            nc.vector.memset(nmb[:, 0:1], 0.0)
            nc.scalar.copy(out=nmb[:, 1:], in_=pnm)
            nc.vector.max(en8, nmb)
            nc.vector.max_index(eni8, en8, nmb)
            nc.vector.tensor_copy(endf, eni8[:, 0:1])

            # active mask act[b, n] = (n <= end_n), transposed to [T, 4]
            nc.vector.tensor_scalar(
                out=act, in0=io129[0:BLOC, :], scalar1=endf, scalar2=None,
                op0=AL.is_le,
            )
            pact = pcb[0:T, 0, 192 : 192 + BLOC]
            ptrans(pact, act[:, 1:])
            nc.scalar.copy(out=actT, in_=pact)

            # y one-hots
            for b in range(BLOC):
                nc.vector.tensor_scalar(
                    out=ybig[:, b, :],
                    in0=io48[0:T, :],
                    scalar1=fidx[:, b : b + 1],
                    scalar2=actT[:, b : b + 1],
                    op0=AL.is_equal,
                    op1=AL.mult,
                )
            nc.sync.dma_start(out=y[:, :, :], in_=ybig)

    nc.finalize()
    _legalize_sync_waits(nc)
    return nc


def _legalize_sync_waits(nc):
    """This container's walrus accepts at most ONE sync wait per instruction.

    Split excess waits onto Drain instructions inserted just before the
    offending instruction (same engine, so the waits still complete before it
    issues; an idle-pipe Drain costs ~12ns).  Applied to the serialized BIR
    only -- CoreSim consumes the in-memory module and is unaffected.
    """
    import json as _json

    m = _json.loads(nc.to_json_bytes())
    for f in m["functions"]:
        for blk in f["blocks"]:
            out = []
            for ins in blk["instructions"]:
                si = ins.get("sync_info") or {}
                w = si.get("on_wait") or []
                if len(w) > 1:
                    for j, wx in enumerate(w[:-1]):
                        out.append(
                            {
                                "debug": ins.get("debug", 0),
                                "engine": ins["engine"],
                                "ins": [],
                                "outs": [],
                                "name": f"{ins['name']}-w{j}",
                                "opcode": "Drain",
                                "sync_info": {"on_update": [], "on_wait": [wx]},
                            }
                        )
                    si["on_wait"] = [w[-1]]
                out.append(ins)
            blk["instructions"] = out
    blob = _json.dumps(m).encode()
    nc.to_json_bytes = lambda: blob


def make_consts():
    f32 = np.float32
    c = np.zeros((128, CW), f32)
    c[0:L, 97] = NEG
    c[0, 97] = 0.0
    d0 = c[0:L, 97].copy()
    c[0:P2, 96] = np.concatenate([d0, d0])
    c[0:L, 98:227] = np.arange(T + 1, dtype=f32)[None, :]
    c[:, 227:275] = np.arange(L, dtype=f32)[None, :]
    return c


def make_in_maps(X, t_feats, e_feats):
    f32 = np.float32
    t_feats = np.asarray(t_feats, dtype=f32)
    e_feats = np.asarray(e_feats, dtype=f32)
    c = make_consts()
    c[0:P2, 0:L] = np.vstack([t_feats, t_feats])
    c[0:L, L : 2 * L] = t_feats.T

    idb = np.eye(128, dtype=f32)

    # e blob [v%128, v//128, 2, L] bf16 (exact hi+lo split of e^T)
    bf16 = mybir.dt.np(BF16)
    eTf = np.zeros((NK * 128, L), f32)
    eTf[:V] = np.ascontiguousarray(e_feats.T)
    ehi = eTf.astype(bf16)
    elo = (eTf - ehi.astype(f32)).astype(bf16)
    efm = np.ascontiguousarray(
        np.stack([ehi, elo], axis=1)
        .reshape(NK, 128, 2, L)
        .transpose(1, 0, 2, 3)
    )

    # x blob per core [v%128, v//128, (b t)] in bf16 (one-hot: exact)
    X = np.asarray(X)
    in_maps = []
    for ci in range(NCORES):
        Xc = np.zeros((BLOC, T, NK * 128), f32)
        Xc[:, :, :V] = X[ci * BLOC : (ci + 1) * BLOC]
        # [b, t, k, p] -> [p, k, b, t]
        xb = np.ascontiguousarray(
            Xc.reshape(BLOC, T, NK, 128).transpose(3, 2, 0, 1).reshape(128, NK, BT)
        ).astype(bf16)
        in_maps.append({"x": xb, "eT": efm, "consts": c, "idb": idb})
    return in_maps


_NC = None


def _get_nc():
    global _NC
    if _NC is None:
        _NC = build_nc()
    return _NC


def kernel(X, t_feats, e_feats):
    in_maps = make_in_maps(X, t_feats, e_feats)
    nc = _get_nc()
    res = run_bass_kernel_spmd(nc, in_maps, list(range(NCORES)))
    out = np.concatenate(
        [res.results[ci]["y"].transpose(1, 0, 2) for ci in range(NCORES)], axis=0
    )
    return np.ascontiguousarray(out, dtype=np.float32)


# revision 25
# speedup vs baseline: 1.3306x; 1.1948x over previous
"""CRF Viterbi decode kernel for Trainium2 (Bass), data-parallel over batch.

Problem shapes (hardcoded): X [32,128,10000] f32 one-hot, t_feats [48,48],
e_feats [48,10000].  Output Y_hat [32,128,48] f32 one-hot.

Sharding: batch 32 -> 8 cores x 4.  t_feats / e_feats replicated.

Per-core pipeline (4 batch elems, labels on 48 partitions throughout):
  1. emissions em[l, (b,t)] = e_feats @ X^T: the host pre-transposes the
     one-hot X into an fp8 [v%128, v//128, (b t)] blob (0/1 are exact in
     fp8), so emissions are just 79 accumulating PE matmuls (f32r e-blob
     stationary x fp8 moving, 512-col outputs) into one PSUM bank -- no
     on-device transposes, no hi/lo split (f32r is exact f32 here), no
     staging copies beyond one PSUM->SBUF copy of the result.
  2. forward Viterbi: per batch elem a PSUM-resident score tile
     psc[j, i] = t[i,j] + delta[i], updated per step with two 48-col
     bf16-identity broadcast matmuls (~20ns each): on-path bcast(m_{p-1})
     and off-path bcast(demf[p-3] - m_{p-2}) (2-step slack).  DVE
     tensor_reduce (per pair of batch elems) produces m_p.  Periodic
     staggered full refreshes bound fp drift.
  3. backward "end-anytime" scan, same structure, with the max over
     next-labels done on the (otherwise idle) Pool engine via a
     mask-reset tensor_tensor_scan: state = max(mask*state, psb[k]);
     the mask zeroes at each batch elem's segment start, which also
     implements beta = max(0, .) for free.  The running-max row is
     written straight into the beta history slab (last col of each
     segment = the true beta).
  4. combine: y_n = onehot(argmax_j delta_n(j)+beta_n(j)) * [n <= end_n]
     via gpsimd partition_all_reduce + elementwise ops; no transposes.
     Output written label-major [L, B, T]; host unscrambles.
"""

import os
import sys

import numpy as np

for _p in ("/opt/trn_rl_repo",):
    if _p not in sys.path and os.path.isdir(_p):
        sys.path.insert(0, _p)

import concourse.bass as bass
import concourse.tile as tile
from concourse import mybir
from concourse.bass_utils import run_bass_kernel_spmd

F32 = mybir.dt.float32
F32R = mybir.dt.float32r
BF16 = mybir.dt.bfloat16
FP8 = mybir.dt.float8e4
AL = mybir.AluOpType
AX = mybir.AxisListType

B, T, V, L = 32, 128, 10000, 48
NCORES = 8
BLOC = B // NCORES          # 4 batch elems per core
NK = 80                     # V chunks of 128 (tail zero-padded; even for DoubleRow)
KP = NK // 2                # DoubleRow processes 2 chunks per matmul
NTERM = 4                   # fp8 e-term count (exact split of e_feats)
BT = BLOC * T               # 512 moving columns, b-major
NEG = -1.0e30
P2 = 2 * L                  # 96

# consts layout [128, CW] f32:
#  cols 0:48     tfS   [96,48]  vstack(t, t)        (fwd refresh lhsT)
#  cols 48:96    tbr   [48,48]  t.T                 (bwd refresh lhsT)
#  col  96       d0stack [96,1]
#  col  97       d0col   [48,1]
#  cols 98:227   iota129 [48,129]
#  cols 227:275  iota48  [128,48]
CW = 275

# fwd refresh schedule: always at p==2 (clears the +-1e30 d0 arithmetic),
# then every 16 steps staggered per pair.  bwd staggered likewise on t.
RF = 16


def _fwd_refresh(p, pair):
    if p == 2:
        return True
    return p > 2 and (p - 2 - 8 * pair) % RF == 0


def _bwd_refresh(t, pair):
    if t == 0:
        return True
    return (t - 6 - 8 * pair) % RF == 0


def build_nc():
    nc = bass.Bass()

    x = nc.declare_dram_parameter("x", [128, KP, 2, BT], FP8, isOutput=False)
    eT = nc.declare_dram_parameter("eT", [128, KP, NTERM, 2, L], FP8, isOutput=False)
    consts = nc.declare_dram_parameter("consts", [128, CW], F32, isOutput=False)
    idb = nc.declare_dram_parameter("idb", [128, 128], F32, isOutput=False)
    y = nc.declare_dram_parameter("y", [T, BLOC, L], F32, isOutput=True)

    with tile.TileContext(nc) as tc:
        from contextlib import ExitStack

        with ExitStack() as ctx:
            cons = ctx.enter_context(tc.tile_pool(name="cons", bufs=1))
            pers = ctx.enter_context(tc.tile_pool(name="pers", bufs=1))
            ppem = ctx.enter_context(tc.tile_pool(name="ppem", bufs=1, space="PSUM"))
            ppfw = ctx.enter_context(tc.tile_pool(name="ppfw", bufs=1, space="PSUM"))
            ppbw = ctx.enter_context(tc.tile_pool(name="ppbw", bufs=1, space="PSUM"))
            ppcb = ctx.enter_context(tc.tile_pool(name="ppcb", bufs=1, space="PSUM"))

            # ---- constants ----
            cons_sb = cons.tile([128, CW], F32)
            nc.sync.dma_start(out=cons_sb, in_=consts[:, :])
            d0col = cons_sb[0:L, 97:98]
            io129 = cons_sb[0:L, 98:227]
            io48 = cons_sb[:, 227:275]

            tfr = cons_sb[0:P2, 0:L]
            tbr = cons_sb[0:L, L : 2 * L]
            d0r = cons_sb[0:P2, 96:97]

            idb_sb = cons.tile([128, 128], F32)
            nc.sync.dma_start(out=idb_sb, in_=idb[:, :])
            id96 = idb_sb[0:P2, 0:P2]
            id48 = idb_sb[0:L, 0:L]

            # ---- persistent state ----
            em48 = pers.tile([L, BLOC, T], F32, name="em48")
            demf = pers.tile([L, BLOC, T - 1], F32, name="demf")
            ndemf = pers.tile([L, BLOC, T - 1], F32, name="ndemf")
            mslab = pers.tile([L, BLOC, T + 1], F32, name="mslab")
            bslab = pers.tile([L, BLOC, T + 2], F32, name="bslab")
            ndm = pers.tile([L, 2, BLOC], F32, name="ndm")
            nbt = pers.tile([L, 2, BLOC], F32, name="nbt")
            dsl = pers.tile([L, BLOC, T + 1], F32, name="dsl")
            fsl = pers.tile([L, BLOC, T + 1], F32, name="fsl")
            fi8 = pers.tile([T, BLOC, 8], mybir.dt.uint32, name="fi8")
            fm8 = pers.tile([T, BLOC, 8], F32, name="fm8")
            fidx = pers.tile([T, BLOC], F32, name="fidx")
            nm = pers.tile([T, BLOC], F32, name="nm")
            nmb = pers.tile([BLOC, T + 1], F32, name="nmb")
            en8 = pers.tile([BLOC, 8], F32, name="en8")
            eni8 = pers.tile([BLOC, 8], mybir.dt.uint32, name="eni8")
            endf = pers.tile([BLOC, 1], F32, name="endf")
            act = pers.tile([BLOC, T + 1], F32, name="act")
            actT = pers.tile([T, BLOC], F32, name="actT")
            ybig = pers.tile([T, BLOC, L], F32, name="ybig")

            pscA = ppfw.tile([L, 2, 512], F32, name="pscA")
            psbA = ppbw.tile([L, 2, 512], F32, name="psbA")
            pem = ppem.tile([L, 2, 512], F32, name="pem")
            # bwd uses 49-wide segments; cols 48/97 stay 0 forever (the
            # max-with-zero floor).  beta_T = 0 (bslab col T).
            nc.vector.memset(psbA[:, :, 48:105:56], 0.0)
            nc.vector.memset(bslab[:, :, T : T + 1], 0.0)

            def bmm(out, col, first=False, last=False, k96=False):
                """Broadcast col ([48,1] or [96,1]) along the free dim of out.

                lhsT/out are bitcast to f32r (same bits as f32 here) so the
                bf16 identity rhs keys the 1.0 cyc/row transpose path.
                """
                kk = P2 if k96 else L
                nc.tensor.matmul(
                    out,
                    col.broadcast_to([kk, L]),
                    id96 if k96 else id48,
                    start=first,
                    stop=last,
                    is_transpose=True,
                    skip_group_check=True,
                )

            def smm(out, lhsT, first=False):
                """Static 96-wide refresh matmul (t-term)."""
                nc.tensor.matmul(
                    out,
                    lhsT,
                    id96,
                    start=first,
                    stop=False,
                    is_transpose=True,
                    skip_group_check=True,
                )

            # ---- emissions: xt/ef live only in this phase; their pool is
            # closed afterwards so the beta slab can reuse the space ----
            epctx = tc.tile_pool(name="ep", bufs=1)
            ep = epctx.__enter__()
            ef = ep.tile([128, KP, NTERM, 2, L], FP8)
            xt = ep.tile([128, KP, 2, BT], FP8)
            KSL = [0, 5, 10, 15, 20, 25, 30, 35, KP]
            for s in range(len(KSL) - 1):
                k0, k1 = KSL[s], KSL[s + 1]
                nc.sync.dma_start(
                    out=ef[:, k0:k1, :, :, :], in_=eT[:, k0:k1, :, :, :]
                )
                nc.sync.dma_start(out=xt[:, k0:k1, :, :], in_=x[:, k0:k1, :, :])
                for k in range(k0, k1):
                    for tm in range(NTERM):
                        # term 0 -> bank A (unscaled); terms 1-3 -> bank B
                        # (stored x256; the combine scales by 2^-8)
                        g = 0 if tm == 0 else 1
                        nc.tensor.matmul(
                            pem[:, g, :],
                            ef[:, k, tm, :, :],
                            xt[:, k, :, :],
                            start=(k == 0 and tm <= 1),
                            stop=(k == KP - 1 and tm in (0, NTERM - 1)),
                            perf_mode=mybir.MatmulPerfMode.DoubleRow,
                        )
            nc.scalar.copy(
                out=em48, in_=pem[:, 0, :].rearrange("p (b t) -> p b t", b=BLOC)
            )
            nc.vector.scalar_tensor_tensor(
                out=em48,
                in0=pem[:, 1, :].rearrange("p (b t) -> p b t", b=BLOC),
                scalar=1.0 / 256.0,
                in1=em48,
                op0=AL.mult,
                op1=AL.add,
            )
            nc.vector.tensor_sub(demf, em48[:, :, 1:], em48[:, :, 0 : T - 1])
            nc.vector.tensor_sub(ndemf, em48[:, :, 0 : T - 1], em48[:, :, 1:])
            epctx.__exit__(None, None, None)  # xt/ef space no longer needed

            # ---- scans ----
            def fwd_step(t):
                p = t + 1
                # off-path bcast term (inputs >= 2 steps old), on Pool
                if p >= 3:
                    q = p % 2
                    nc.gpsimd.tensor_tensor(
                        ndm[:, q, :],
                        demf[:, :, p - 3],
                        mslab[:, :, p - 2],
                        op=AL.subtract,
                    )
                for pair in range(2):
                    ps = pscA[:, pair, :]
                    if p == 1:
                        smm(ps[:, 0:P2], tfr, first=True)
                        bmm(ps[:, 0:P2], d0r, k96=True)
                    elif _fwd_refresh(p, pair):
                        smm(ps[:, 0:P2], tfr, first=True)
                        for sub in range(2):
                            b = 2 * pair + sub
                            blk = ps[:, L * sub : L * sub + L]
                            bmm(blk, em48[:, b, p - 2 : p - 1])
                            bmm(blk, mslab[:, b, p - 1 : p])
                    else:
                        q = p % 2
                        for sub in range(2):
                            b = 2 * pair + sub
                            blk = ps[:, L * sub : L * sub + L]
                            bmm(blk, mslab[:, b, p - 1 : p])
                            bmm(blk, ndm[:, q, b : b + 1])
                nc.vector.tensor_reduce(
                    out=mslab[:, :, p : p + 1],
                    in_=pscA[:, :, 0:P2].rearrange("p a (s i) -> p a s i", s=2),
                    axis=AX.X,
                    op=AL.max,
                )

            def bwd_step(t):
                pos = T - 1 - t
                if pos <= T - 2:
                    # nb_pos = -demf[pos] - beta_{pos+2} (off-path, Pool)
                    q = t % 2
                    nc.gpsimd.tensor_tensor(
                        nbt[:, q, :],
                        ndemf[:, :, pos],
                        bslab[:, :, pos + 2],
                        op=AL.subtract,
                    )
                for pair in range(2):
                    pb = psbA[:, pair, :]
                    rf = _bwd_refresh(t, pair)
                    for sub in range(2):
                        b = 2 * pair + sub
                        blk = pb[:, 56 * sub : 56 * sub + L]
                        if rf:
                            nc.tensor.matmul(
                                blk,
                                tbr,
                                id48,
                                start=(sub == 0),
                                stop=False,
                                is_transpose=True,
                                skip_group_check=True,
                            )
                            bmm(blk, em48[:, b, pos : pos + 1])
                            if t > 0:
                                bmm(blk, bslab[:, b, pos + 1 : pos + 2])
                        else:
                            q = t % 2
                            bmm(blk, bslab[:, b, pos + 1 : pos + 2])
                            bmm(blk, nbt[:, q, b : b + 1])
                nc.vector.tensor_reduce(
                    out=bslab[:, :, pos : pos + 1],
                    in_=psbA[:, :, 0:112].rearrange("p a (s i) -> p a s i", i=56)[
                        :, :, :, 0:49
                    ],
                    axis=AX.X,
                    op=AL.max,
                )

            # middle combine-sum columns [C0, C1) become ready mid-loop
            # (mslab fills forward, bslab backward: col n ready at
            # t = max(n-1, T-1-n))
            C0, C1 = 8, 122
            for t in range(T):
                fwd_step(t)
                bwd_step(t)
                if t == T - 8:
                    nc.vector.tensor_add(
                        dsl[:, :, C0:C1], mslab[:, :, C0:C1], em48[:, :, C0 - 1 : C1 - 1]
                    )
                    nc.vector.tensor_add(
                        fsl[:, :, C0:C1], dsl[:, :, C0:C1], bslab[:, :, C0:C1]
                    )

            # ---- combine ----
            id128f = idb_sb

            def ptrans(out_psum, in_sb):
                nc.tensor.matmul(
                    out_psum,
                    in_sb,
                    id128f[0 : in_sb.shape[0], 0 : in_sb.shape[0]],
                    start=True,
                    stop=True,
                    is_transpose=True,
                    skip_group_check=True,
                )

            # dsl: col0 = d0, cols 1: = m + em;  fsl = dsl + beta
            # (middle columns were computed during the scan loop)
            nc.vector.tensor_add(
                dsl[:, :, 1:C0], mslab[:, :, 1:C0], em48[:, :, 0 : C0 - 1]
            )
            nc.vector.tensor_add(
                dsl[:, :, C1:], mslab[:, :, C1:], em48[:, :, C1 - 1 :]
            )
            nc.vector.tensor_copy(dsl[:, :, 0:1], d0col.broadcast_to([L, BLOC, 1]))
            nc.vector.tensor_add(
                fsl[:, :, 0:C0], dsl[:, :, 0:C0], bslab[:, :, 0:C0]
            )
            nc.vector.tensor_add(
                fsl[:, :, C1:], dsl[:, :, C1:], bslab[:, :, C1 : T + 1]
            )

            # per-b transposes to [T, 48] (fresh PSUM banks) + max/argmax
            pcb = ppcb.tile([128, 2, 512], F32, name="pcb")
            for b in range(BLOC):
                ptd = pcb[:, 0, b * L : (b + 1) * L][0:T, :]
                ptf = pcb[:, 1, b * L : (b + 1) * L][0:T, :]
                ptrans(ptd, dsl[:, b, 1:])
                ptrans(ptf, fsl[:, b, 1:])
            nc.vector.tensor_reduce(
                out=nm,
                in_=pcb[0:T, 0, 0 : BLOC * L].rearrange("p (b l) -> p b l", b=BLOC),
                axis=AX.X,
                op=AL.max,
            )
            for b in range(BLOC):
                ptf = pcb[:, 1, b * L : (b + 1) * L][0:T, :]
                nc.vector.max(fm8[:, b, :], ptf)
                nc.vector.max_index(fi8[:, b, :], fm8[:, b, :], ptf)
            nc.vector.tensor_copy(fidx, fi8[:, :, 0])

            # end_n per b from n_maxs (col 0 of delta_full maxes to 0)
            pnm = pcb[0:BLOC, 1, 192 : 192 + T]
            ptrans(pnm, nm)
            nc.vector.memset(nmb[:, 0:1], 0.0)
            nc.scalar.copy(out=nmb[:, 1:], in_=pnm)
            nc.vector.max(en8, nmb)
            nc.vector.max_index(eni8, en8, nmb)
            nc.vector.tensor_copy(endf, eni8[:, 0:1])

            # active mask act[b, n] = (n <= end_n), transposed to [T, 4]
            nc.vector.tensor_scalar(
                out=act, in0=io129[0:BLOC, :], scalar1=endf, scalar2=None,
                op0=AL.is_le,
            )
            pact = pcb[0:T, 0, 192 : 192 + BLOC]
            ptrans(pact, act[:, 1:])
            nc.scalar.copy(out=actT, in_=pact)

            # y one-hots
            for b in range(BLOC):
                nc.vector.tensor_scalar(
                    out=ybig[:, b, :],
                    in0=io48[0:T, :],
                    scalar1=fidx[:, b : b + 1],
                    scalar2=actT[:, b : b + 1],
                    op0=AL.is_equal,
                    op1=AL.mult,
                )
            nc.sync.dma_start(out=y[:, :, :], in_=ybig)

    nc.finalize()
    _legalize_sync_waits(nc)
    return nc


def _legalize_sync_waits(nc):
    """This container's walrus accepts at most ONE sync wait per instruction.

    Split excess waits onto Drain instructions inserted just before the
    offending instruction (same engine, so the waits still complete before it
    issues; an idle-pipe Drain costs ~12ns).  Applied to the serialized BIR
    only -- CoreSim consumes the in-memory module and is unaffected.
    """
    import json as _json

    m = _json.loads(nc.to_json_bytes())
    for f in m["functions"]:
        for blk in f["blocks"]:
            out = []
            for ins in blk["instructions"]:
                si = ins.get("sync_info") or {}
                w = si.get("on_wait") or []
                if len(w) > 1:
                    for j, wx in enumerate(w[:-1]):
                        out.append(
                            {
                                "debug": ins.get("debug", 0),
                                "engine": ins["engine"],
                                "ins": [],
                                "outs": [],
                                "name": f"{ins['name']}-w{j}",
                                "opcode": "Drain",
                                "sync_info": {"on_update": [], "on_wait": [wx]},
                            }
                        )
                    si["on_wait"] = [w[-1]]
                out.append(ins)
            blk["instructions"] = out
    blob = _json.dumps(m).encode()
    nc.to_json_bytes = lambda: blob


def make_consts():
    f32 = np.float32
    c = np.zeros((128, CW), f32)
    c[0:L, 97] = NEG
    c[0, 97] = 0.0
    d0 = c[0:L, 97].copy()
    c[0:P2, 96] = np.concatenate([d0, d0])
    c[0:L, 98:227] = np.arange(T + 1, dtype=f32)[None, :]
    c[:, 227:275] = np.arange(L, dtype=f32)[None, :]
    return c


def make_in_maps(X, t_feats, e_feats):
    f32 = np.float32
    t_feats = np.asarray(t_feats, dtype=f32)
    e_feats = np.asarray(e_feats, dtype=f32)
    c = make_consts()
    c[0:P2, 0:L] = np.vstack([t_feats, t_feats])
    c[0:L, L : 2 * L] = t_feats.T

    idb = np.eye(128, dtype=f32)

    # e blob [v%128, kp, term, j, L] fp8: 4-term exact-to-~2^-16 split of e^T
    fp8 = mybir.dt.np(FP8)
    eTf = np.zeros((NK * 128, L), f32)
    eTf[:V] = np.ascontiguousarray(e_feats.T)
    terms = []
    t0 = eTf.astype(fp8)
    terms.append(t0)
    rs = (eTf - t0.astype(f32)) * 256.0
    for _ in range(NTERM - 1):
        t = rs.astype(fp8)
        terms.append(t)
        rs = rs - t.astype(f32)
    efm = np.ascontiguousarray(
        np.stack(terms, axis=1)              # [NK*128, NTERM, L]
        .reshape(KP, 2, 128, NTERM, L)       # [kp, j, p, term, L]
        .transpose(2, 0, 3, 1, 4)            # [p, kp, term, j, L]
    )

    # x blob per core [v%128, kp, j, (b t)] in fp8 (one-hot: exact)
    X = np.asarray(X)
    in_maps = []
    for ci in range(NCORES):
        Xc = np.zeros((BLOC, T, NK * 128), f32)
        Xc[:, :, :V] = X[ci * BLOC : (ci + 1) * BLOC]
        # [b, t, kp, j, p] -> [p, kp, j, b, t]
        xb = np.ascontiguousarray(
            Xc.reshape(BLOC, T, KP, 2, 128)
            .transpose(4, 2, 3, 0, 1)
            .reshape(128, KP, 2, BT)
        ).astype(fp8)
        in_maps.append({"x": xb, "eT": efm, "consts": c, "idb": idb})
    return in_maps


_NC = None


def _get_nc():
    global _NC
    if _NC is None:
        _NC = build_nc()
    return _NC


def kernel(X, t_feats, e_feats):
    in_maps = make_in_maps(X, t_feats, e_feats)
    nc = _get_nc()
    res = run_bass_kernel_spmd(nc, in_maps, list(range(NCORES)))
    out = np.concatenate(
        [res.results[ci]["y"].transpose(1, 0, 2) for ci in range(NCORES)], axis=0
    )
    return np.ascontiguousarray(out, dtype=np.float32)
